# revision 2
# baseline (speedup 1.0000x reference)
"""DeepGCN (4-layer GCN, N=50000, E=800000, D=128) on 8 Trainium2 cores.

v2: feature-major on-chip layout, bf16 spmm path, host-precomputed one-hot
M matrices streamed from DRAM (no DVE is_eq builds), Shared AllGather.

Per layer (per core):
 - dma_gather source rows of X (bf16, DRAM, replicated via AllGather) in
   edge-chunk order; lo/hi split for the int16 index limit.
 - zT_s[feat, dst] = sum_k G_k^T M_k on the PE (bf16 inputs, fp32 PSUM);
   M is the host-built one-hot*val matrix, streamed from DRAM.
 - hT = W_l^T zb in 13 N=512 matmuls (weights stationary).
 - PairNorm stats as two big-tile ops + [P,2] AllReduce; pass2 as one
   activation + one add over [128, 6272].
 - Write-back: 49 PE transposes -> bf16 stage -> one DMA -> AllGather
   into the next layer's Shared X table.
"""

import sys

sys.path.insert(0, "/opt/trn_rl_repo")

import ml_dtypes
import numpy as np

import concourse.bacc as bacc
import concourse.mybir as mybir
import concourse.tile as tile
from concourse.bass_utils import run_bass_kernel_spmd
from concourse.library_config import mlp
from concourse.masks import make_identity

P = 128
NCORES = 8
N = 50000
D = 128
C = 40
L = 4
SLOTS = 49
NS = SLOTS * P
NTOT = NCORES * NS
LO_LIMIT = 32768
MAXCH = 8
EPS_BN = 1e-5
EPS_PN = 1e-6
NFULL = SLOTS * 4  # 13 chunks of 512 would overrun; use 512-col chunks below
GCH = 512
NGC = NS // GCH  # 12 full chunks of 512
GREM = NS - NGC * GCH  # remainder 128

F32 = mybir.dt.float32
BF16 = mybir.dt.bfloat16
I16 = mybir.dt.int16
I32 = mybir.dt.int32

TRACE = False
LAST_EXEC_NS = None
LAST_TRACE = None
DEBUG_DUMP = False

_nc_cache = {}


# ------------------------------------------------------------------ host prep

def _positions(edge_row):
    """Assign nodes to (core, slot, offset) balancing in-degree."""
    deg = np.bincount(edge_row, minlength=N)
    order = np.argsort(-deg, kind="stable")
    r = np.arange(N)
    rnd, pc = r // NCORES, r % NCORES
    core_of_rank = np.where(rnd % 2 == 0, pc, NCORES - 1 - pc)

    pos = np.empty(N, np.int64)
    for c in range(NCORES):
        nodes_c = order[core_of_rank == c]
        m = len(nodes_c)
        rr = np.arange(m)
        rnd2, ps_ = rr // SLOTS, rr % SLOTS
        slot = np.where(rnd2 % 2 == 0, ps_, SLOTS - 1 - ps_)
        off = rnd2
        pos[nodes_c] = c * NS + slot * P + off

    pos2node = np.full(NCORES * NS, -1, np.int64)
    pos2node[pos] = np.arange(N)
    return pos, pos2node


def _preprocess(edge_row, edge_col, edge_val):
    pos, pos2node = _positions(edge_row)
    pd = pos[edge_row]
    ps = pos[edge_col]
    core = pd // NS
    slotg = (pd % NS) // P
    doff = pd % P
    hi = (ps >= LO_LIMIT).astype(np.int64)
    gi = (ps - hi * LO_LIMIT).astype(np.int64)

    key3 = (core * SLOTS + slotg) * 2 + hi
    cnt = np.bincount(key3, minlength=NCORES * SLOTS * 2).reshape(
        NCORES, SLOTS, 2)
    K_LO = np.ceil(cnt[:, :, 0].max(axis=0) / P).astype(int)
    K_HI = np.ceil(cnt[:, :, 1].max(axis=0) / P).astype(int)

    base_lo = np.zeros(SLOTS, int)
    base_hi = np.zeros(SLOTS, int)
    ctr = 0
    for s in range(SLOTS):
        base_lo[s] = ctr
        ctr += K_LO[s]
        base_hi[s] = ctr
        ctr += K_HI[s]
    TOT = ctr
    sb_lo = np.concatenate([[0], np.cumsum(K_LO)[:-1]])
    sb_hi = np.concatenate([[0], np.cumsum(K_HI)[:-1]])
    KLT, KHT = int(K_LO.sum()), int(K_HI.sum())

    per_core = []
    for c in range(NCORES):
        sel = np.flatnonzero(core == c)
        k = slotg[sel] * 2 + hi[sel]
        si = np.argsort(k, kind="stable")
        es = sel[si]
        ks = k[si]
        m = len(es)
        change = np.r_[True, np.diff(ks) != 0]
        segstart = np.maximum.accumulate(np.where(change, np.arange(m), 0))
        rank = np.arange(m) - segstart

        m_full = np.zeros((P, max(TOT, 1) * P), np.float32)
        idx_lo_flat = np.zeros(max(KLT, 1) * P, np.int16)
        idx_hi_flat = np.zeros(max(KHT, 1) * P, np.int16)

        for is_hi, base, sbase, flat in (
            (0, base_lo, sb_lo, idx_lo_flat),
            (1, base_hi, sb_hi, idx_hi_flat),
        ):
            msk = hi[es] == is_hi
            ee = es[msk]
            rk = rank[msk]
            sl = slotg[ee]
            gch = base[sl] + rk // P
            m_full[rk % P, gch * P + doff[ee]] = edge_val[ee]
            flat[sbase[sl] * P + rk] = gi[ee]

        def wrap(flat, kt):
            a = flat.reshape(kt * 8, 16).T
            return np.ascontiguousarray(np.tile(a, (8, 1)))

        per_core.append(dict(
            m_full=m_full.astype(ml_dtypes.bfloat16),
            idx_lo=wrap(idx_lo_flat, max(KLT, 1)),
            idx_hi=wrap(idx_hi_flat, max(KHT, 1)),
        ))

    sched = (tuple(int(x) for x in K_LO), tuple(int(x) for x in K_HI))
    meta = dict(K_LO=K_LO, K_HI=K_HI, base_lo=base_lo, base_hi=base_hi,
                sb_lo=sb_lo, sb_hi=sb_hi, TOT=TOT, KLT=KLT, KHT=KHT)
    return pos, pos2node, per_core, sched, meta


# ------------------------------------------------------------------ bass build

def _build(meta):
    K_LO, K_HI = meta["K_LO"], meta["K_HI"]
    base_lo, base_hi = meta["base_lo"], meta["base_hi"]
    sb_lo, sb_hi = meta["sb_lo"], meta["sb_hi"]
    TOT, KLT, KHT = meta["TOT"], meta["KLT"], meta["KHT"]
    OP = mybir.AluOpType
    AF = mybir.ActivationFunctionType
    AX = mybir.AxisListType
    RG = [list(range(NCORES))]

    nc = bacc.Bacc("TRN2", target_bir_lowering=False, debug=False,
                   num_devices=NCORES)

    xt_own = nc.dram_tensor("xt_own", [P, NS], F32, kind="ExternalInput")
    idx_lo = nc.dram_tensor("idx_lo", [P, max(KLT, 1) * 8], I16,
                            kind="ExternalInput")
    idx_hi = nc.dram_tensor("idx_hi", [P, max(KHT, 1) * 8], I16,
                            kind="ExternalInput")
    m_full = nc.dram_tensor("m_full", [P, max(TOT, 1) * P], BF16,
                            kind="ExternalInput")
    fc_in_w = nc.dram_tensor("fc_in_w", [D, D], F32, kind="ExternalInput")
    fc_in_bT = nc.dram_tensor("fc_in_bT", [D, 1], F32, kind="ExternalInput")
    bn_g = nc.dram_tensor("bn_g", [1, D], F32, kind="ExternalInput")
    bn_b = nc.dram_tensor("bn_b", [1, D], F32, kind="ExternalInput")
    gc_w = nc.dram_tensor("gc_w", [L * D, D], BF16, kind="ExternalInput")
    fc_out_w = nc.dram_tensor("fc_out_w", [D, C], BF16, kind="ExternalInput")
    fc_out_bT = nc.dram_tensor("fc_out_bT", [C, 1], F32,
                               kind="ExternalInput")
    out = nc.dram_tensor("out", [NS, C], F32, kind="ExternalOutput")
    if DEBUG_DUMP:
        dbg_xn = nc.dram_tensor("dbg_xn", [P, NS], F32, kind="ExternalOutput")
        dbg_h = nc.dram_tensor("dbg_h", [P, NS], F32, kind="ExternalOutput")
        dbg_z = nc.dram_tensor("dbg_z", [P, NS], BF16, kind="ExternalOutput")

    with tile.TileContext(nc) as tc:
        nc.gpsimd.load_library(mlp)
        with (
            tc.tile_pool(name="const", bufs=1) as cp,
            tc.tile_pool(name="meta", bufs=1) as mp_,
            tc.tile_pool(name="big", bufs=1) as bp,
            tc.tile_pool(name="gpool", bufs=2) as gp,
            tc.tile_pool(name="mpool", bufs=3) as mpl,
            tc.tile_pool(name="small", bufs=1) as sp,
            tc.tile_pool(name="dram", bufs=1, space="DRAM") as dp,
        ):
            # ---------------- constants
            ident = cp.tile([P, P], F32)
            make_identity(nc, ident[:])
            ones_col = cp.tile([P, 1], F32)
            nc.vector.memset(ones_col[:], 1.0)
            ones_row = cp.tile([1, P], F32)
            nc.vector.memset(ones_row[:], 1.0)
            eps_bn_t = cp.tile([P, 1], F32)
            nc.vector.memset(eps_bn_t[:], EPS_BN)
            eps_pn_t = cp.tile([1, 1], F32)
            nc.vector.memset(eps_pn_t[:], EPS_PN)

            w1_raw = cp.tile([D, D], F32)
            nc.sync.dma_start(w1_raw[:], fc_in_w[:])
            fcbT_s = cp.tile([D, 1], F32)
            nc.sync.dma_start(fcbT_s[:], fc_in_bT[:])
            bn_s = cp.tile([2, D], F32)
            nc.sync.dma_start(bn_s[0:1, :], bn_g[:])
            nc.sync.dma_start(bn_s[1:2, :], bn_b[:])
            gw_s = [cp.tile([D, D], BF16, tag=f"gw{i}", name=f"gw{i}")
                    for i in range(L)]
            for i in range(L):
                nc.sync.dma_start(gw_s[i][:], gc_w[i * D:(i + 1) * D, :])
            wo_s = cp.tile([D, C], BF16)
            nc.sync.dma_start(wo_s[:], fc_out_w[:])
            boT_s = cp.tile([C, 1], F32)
            nc.sync.dma_start(boT_s[:], fc_out_bT[:])

            idx_lo_s = mp_.tile([P, max(KLT, 1) * 8], I16)
            nc.sync.dma_start(idx_lo_s[:], idx_lo[:])
            idx_hi_s = mp_.tile([P, max(KHT, 1) * 8], I16)
            nc.sync.dma_start(idx_hi_s[:], idx_hi[:])

            # ---------------- big SBUF tiles (feature-major)
            hsb = bp.tile([P, NS], F32)     # h^T for the current layer
            xn = bp.tile([P, NS], F32)      # residual state x^T
            rbuf = bp.tile([P, NS], F32)    # relu temp
            zb = bp.tile([P, NS], BF16)     # z^T (spmm result)
            stage = bp.tile([P, SLOTS, P], BF16)  # row-major write-back

            # DRAM internals (Shared: written once each, read by all cores)
            X_t = [dp.tile([NTOT, P], BF16, addr_space="Shared",
                           tag=f"X{i}", name=f"X{i}")
                   for i in range(L)]
            ag_in = dp.tile([NS, P], BF16)
            st_in = dp.tile([P, 2], F32)
            st_out = dp.tile([P, 2], F32)

            def row_major_writeback(src_big, psp, tag):
                """src_big [P(feat), NS] f32 -> stage bf16 -> ag_in DRAM."""
                for s in range(SLOTS):
                    tp = psp.tile([P, P], F32, space="PSUM", tag=tag)
                    nc.tensor.transpose(
                        tp[:], src_big[:, s * P:(s + 1) * P], ident[:])
                    nc.scalar.copy(stage[:, s, :], tp[:])
                nc.sync.dma_start(
                    ag_in[:].rearrange("(s d) f -> d s f", s=SLOTS),
                    stage[:])

            # ---------------- phase 0: BN stats + folded fc_in
            with (
                tc.tile_pool(name="p0psum", bufs=2, space="PSUM") as pp0,
                tc.tile_pool(name="p0sb", bufs=1) as sp0,
            ):
                xt_s = sp0.tile([P, NS], F32)
                nc.sync.dma_start(xt_s[:], xt_own[:])

                colsum_o = sp0.tile([P, 1], F32)
                sumsq_o = sp0.tile([P, 1], F32)
                scratch = sp0.tile([P, NS], F32)
                nc.vector.tensor_reduce(colsum_o[:], xt_s[:], axis=AX.X,
                                        op=OP.add)
                nc.scalar.activation(scratch[:], xt_s[:], AF.Square,
                                     accum_out=sumsq_o[:])
                st2 = sp0.tile([P, 2], F32)
                nc.vector.tensor_copy(st2[:, 0:1], colsum_o[:])
                nc.vector.tensor_copy(st2[:, 1:2], sumsq_o[:])
                nc.sync.dma_start(st_in[:], st2[:])
                nc.gpsimd.collective_compute(
                    "AllReduce", OP.add, replica_groups=RG,
                    ins=[st_in[:]], outs=[st_out[:]])
                stg = sp0.tile([P, 2], F32)
                nc.sync.dma_start(stg[:], st_out[:])

                mu = sp0.tile([P, 1], F32)
                nc.vector.tensor_scalar_mul(mu[:], stg[:, 0:1], 1.0 / N)
                msq = sp0.tile([P, 1], F32)
                nc.vector.tensor_scalar_mul(msq[:], stg[:, 1:2], 1.0 / N)
                mu2 = sp0.tile([P, 1], F32)
                nc.vector.tensor_tensor(mu2[:], mu[:], mu[:], op=OP.mult)
                var = sp0.tile([P, 1], F32)
                nc.vector.tensor_tensor(var[:], msq[:], mu2[:],
                                        op=OP.subtract)
                sd = sp0.tile([P, 1], F32)
                nc.scalar.activation(sd[:], var[:], AF.Sqrt,
                                     bias=eps_bn_t[:])
                rs = sp0.tile([P, 1], F32)
                nc.vector.reciprocal(rs[:], sd[:])

                bnT_ps = pp0.tile([P, 2], F32, space="PSUM", tag="pp0a")
                nc.tensor.transpose(bnT_ps[:], bn_s[:], ident[:2, :2])
                bnT = sp0.tile([P, 2], F32)
                nc.scalar.copy(bnT[:], bnT_ps[:])
                a_t = sp0.tile([P, 1], F32)
                nc.vector.tensor_tensor(a_t[:], bnT[:, 0:1], rs[:],
                                        op=OP.mult)
                t2 = sp0.tile([P, 1], F32)
                nc.vector.tensor_tensor(t2[:], mu[:], a_t[:], op=OP.mult)
                csh = sp0.tile([P, 1], F32)
                nc.vector.tensor_tensor(csh[:], bnT[:, 1:2], t2[:],
                                        op=OP.subtract)
                W1f = sp0.tile([D, D], F32)
                nc.vector.tensor_scalar_mul(W1f[:], w1_raw[:], a_t[:])
                # b1T = W1_raw^T @ csh + fc_in_bT   [D, 1]
                b1_ps = pp0.tile([P, 1], F32, space="PSUM", tag="pp0a")
                nc.tensor.matmul(b1_ps[:], lhsT=w1_raw[:], rhs=csh[:],
                                 start=True, stop=True)
                b1T = sp0.tile([P, 1], F32)
                nc.scalar.copy(b1T[:], b1_ps[:])
                nc.vector.tensor_tensor(b1T[:], b1T[:], fcbT_s[:], op=OP.add)

                # x0^T = W1f^T @ x^T + b1T  (chunks of 512)
                for j in range(NGC + 1):
                    c0 = j * GCH
                    w = GCH if j < NGC else GREM
                    if w == 0:
                        break
                    g_ps = pp0.tile([P, GCH], F32, space="PSUM", tag="g0")
                    nc.tensor.matmul(g_ps[:, :w], lhsT=W1f[:],
                                     rhs=xt_s[:, c0:c0 + w],
                                     start=True, stop=True)
                    nc.vector.tensor_scalar(
                        out=xn[:, c0:c0 + w], in0=g_ps[:, :w],
                        scalar1=b1T[:], scalar2=None, op0=OP.add)
                row_major_writeback(xn, pp0, "g0")
                nc.gpsimd.collective_compute(
                    "AllGather", OP.bypass, replica_groups=RG,
                    ins=[ag_in[:]], outs=[X_t[0][:]])

            # ---------------- layers
            for li in range(L):
                XIN = X_t[li]
                with (
                    tc.tile_pool(name=f"l{li}ps", bufs=2, space="PSUM") as lp,
                    tc.tile_pool(name=f"l{li}hp", bufs=2, space="PSUM") as hp,
                ):
                    for s in range(SLOTS):
                        klo, khi = int(K_LO[s]), int(K_HI[s])
                        ng = klo + khi
                        Gt = gp.tile([P, ng, P], BF16, tag="G")
                        for b0 in range(0, klo, MAXCH):
                            kk = min(MAXCH, klo - b0)
                            c0 = int(sb_lo[s]) + b0
                            nc.gpsimd.dma_gather(
                                Gt[:, b0:b0 + kk, :], XIN[:],
                                idx_lo_s[:, c0 * 8:(c0 + kk) * 8],
                                kk * P, kk * P, P)
                        for b0 in range(0, khi, MAXCH):
                            kk = min(MAXCH, khi - b0)
                            c0 = int(sb_hi[s]) + b0
                            nc.gpsimd.dma_gather(
                                Gt[:, klo + b0:klo + b0 + kk, :],
                                XIN[LO_LIMIT:, :],
                                idx_hi_s[:, c0 * 8:(c0 + kk) * 8],
                                kk * P, kk * P, P)
                        Mt = mpl.tile([P, ng, P], BF16, tag="M")
                        mc0 = int(base_lo[s])
                        nc.sync.dma_start(
                            Mt[:], m_full[:, mc0 * P:(mc0 + ng) * P])

                        zT = lp.tile([P, P], F32, space="PSUM", tag="zT")
                        blocks = (
                            [(j, int(base_lo[s]) - mc0 + j)
                             for j in range(klo)] +
                            [(klo + j, int(base_hi[s]) - mc0 + j)
                             for j in range(khi)])
                        for j, (blk, mblk) in enumerate(blocks):
                            nc.tensor.matmul(
                                zT[:], lhsT=Gt[:, blk, :],
                                rhs=Mt[:, mblk, :],
                                start=(j == 0),
                                stop=(j == len(blocks) - 1))
                        if s % 2 == 0:
                            nc.scalar.copy(zb[:, s * P:(s + 1) * P], zT[:])
                        else:
                            nc.vector.tensor_copy(
                                zb[:, s * P:(s + 1) * P], zT[:])

                    # hT = W_l^T @ zb  (chunks of 512)
                    for j in range(NGC + 1):
                        c0 = j * GCH
                        w = GCH if j < NGC else GREM
                        if w == 0:
                            break
                        h_ps = hp.tile([P, GCH], F32, space="PSUM", tag="h")
                        nc.tensor.matmul(h_ps[:, :w], lhsT=gw_s[li][:],
                                         rhs=zb[:, c0:c0 + w],
                                         start=True, stop=True)
                        nc.scalar.copy(hsb[:, c0:c0 + w], h_ps[:, :w])

                    # PairNorm stats
                    colsum = sp.tile([P, 1], F32, tag="colsum")
                    nc.vector.tensor_reduce(colsum[:], hsb[:], axis=AX.X,
                                            op=OP.add)
                    sumsq = sp.tile([P, 1], F32, tag="sumsq")
                    nc.scalar.activation(rbuf[:], hsb[:], AF.Square,
                                         accum_out=sumsq[:])
                    st2l = sp.tile([P, 2], F32, tag="st2l")
                    nc.vector.tensor_copy(st2l[:, 0:1], colsum[:])
                    nc.vector.tensor_copy(st2l[:, 1:2], sumsq[:])
                    nc.sync.dma_start(st_in[:], st2l[:])
                    nc.gpsimd.collective_compute(
                        "AllReduce", OP.add, replica_groups=RG,
                        ins=[st_in[:]], outs=[st_out[:]])
                    stgl = sp.tile([P, 2], F32, tag="stgl")
                    nc.sync.dma_start(stgl[:], st_out[:])

                    cmean = sp.tile([P, 1], F32, tag="cmean")
                    nc.vector.tensor_scalar_mul(cmean[:], stgl[:, 0:1],
                                                1.0 / N)
                    csq = sp.tile([P, 1], F32, tag="csq")
                    nc.vector.tensor_tensor(csq[:], stgl[:, 0:1],
                                            stgl[:, 0:1], op=OP.mult)
                    nc.vector.tensor_scalar_mul(csq[:], csq[:], 1.0 / N)
                    q = sp.tile([P, 1], F32, tag="q")
                    nc.vector.tensor_tensor(q[:], stgl[:, 1:2], csq[:],
                                            op=OP.subtract)
                    tot_ps = lp.tile([1, 1], F32, space="PSUM", tag="tot")
                    nc.tensor.matmul(tot_ps[:], lhsT=q[:], rhs=ones_col[:],
                                     start=True, stop=True)
                    tot_s = sp.tile([1, 1], F32, tag="tot")
                    nc.scalar.copy(tot_s[:], tot_ps[:])
                    rn = sp.tile([1, 1], F32, tag="rn")
                    nc.scalar.activation(rn[:], tot_s[:], AF.Sqrt,
                                         bias=eps_pn_t[:], scale=1.0 / N)
                    sres = sp.tile([1, 1], F32, tag="sres")
                    nc.vector.reciprocal(sres[:], rn[:])
                    sbc_ps = lp.tile([P, 1], F32, space="PSUM", tag="sbc")
                    nc.tensor.matmul(sbc_ps[:], lhsT=ones_row[:],
                                     rhs=sres[:], start=True, stop=True)
                    sbc = sp.tile([P, 1], F32, tag="sbc")
                    nc.scalar.copy(sbc[:], sbc_ps[:])
                    nsm = sp.tile([P, 1], F32, tag="nsm")
                    nc.vector.tensor_tensor(nsm[:], cmean[:], sbc[:],
                                            op=OP.mult)
                    nc.vector.tensor_scalar_mul(nsm[:], nsm[:], -1.0)

                    # pass 2: xn = relu(s*h - s*mean) + x_old
                    # (x_old starts at ZERO: no residual add at layer 0)
                    if li == 0:
                        nc.scalar.activation(xn[:], hsb[:], AF.Relu,
                                             scale=sbc[:], bias=nsm[:])
                    else:
                        nc.scalar.activation(rbuf[:], hsb[:], AF.Relu,
                                             scale=sbc[:], bias=nsm[:])
                        nc.vector.tensor_tensor(xn[:], rbuf[:], xn[:],
                                                op=OP.add)

                    if li < L - 1:
                        row_major_writeback(xn, lp, "zT")
                        nc.gpsimd.collective_compute(
                            "AllGather", OP.bypass, replica_groups=RG,
                            ins=[ag_in[:]], outs=[X_t[li + 1][:]])

            if DEBUG_DUMP:
                nc.sync.dma_start(dbg_xn[:], xn[:])
                nc.sync.dma_start(dbg_h[:], hsb[:])
                nc.sync.dma_start(dbg_z[:], zb[:])

            # ---------------- fc_out
            with (
                tc.tile_pool(name="fo", bufs=3, space="PSUM") as fp,
                tc.tile_pool(name="fos", bufs=2) as fs,
            ):
                xnb = fs.tile([P, NS], BF16)
                nc.vector.tensor_copy(xnb[:], xn[:])
                for j in range(NGC + 1):
                    c0 = j * GCH
                    w = GCH if j < NGC else GREM
                    if w == 0:
                        break
                    o_ps = fp.tile([C, GCH], F32, space="PSUM", tag="o")
                    nc.tensor.matmul(o_ps[:, :w], lhsT=wo_s[:],
                                     rhs=xnb[:, c0:c0 + w],
                                     start=True, stop=True)
                    oT = fs.tile([C, GCH], F32, tag="oT")
                    nc.vector.tensor_scalar(
                        out=oT[:, :w], in0=o_ps[:, :w],
                        scalar1=boT_s[:], scalar2=None, op0=OP.add)
                    for d in range(w // P):
                        s = (c0 + d * P) // P
                        tp_ps = fp.tile([P, C], F32, space="PSUM", tag="tp")
                        nc.tensor.transpose(
                            tp_ps[:], oT[:, d * P:(d + 1) * P],
                            ident[:C, :C])
                        o_s = fs.tile([P, C], F32, tag="os")
                        nc.scalar.copy(o_s[:], tp_ps[:])
                        nc.sync.dma_start(out[s * P:(s + 1) * P, :], o_s[:])

    nc.compile()
    return nc


# ------------------------------------------------------------------ kernel

def kernel(x, edge_row, edge_col, edge_val, bn_gamma, bn_beta,
           fc_in_w, fc_in_b, gc_w, gc_b, fc_out_w, fc_out_b):
    global LAST_EXEC_NS, LAST_TRACE
    x = np.asarray(x, np.float32)
    edge_row = np.asarray(edge_row).astype(np.int64)
    edge_col = np.asarray(edge_col).astype(np.int64)
    edge_val = np.asarray(edge_val, np.float32)

    pos, pos2node, per_core, sched, meta = _preprocess(
        edge_row, edge_col, edge_val)

    if sched not in _nc_cache:
        _nc_cache[sched] = _build(meta)
    nc = _nc_cache[sched]

    x_pad = np.zeros((NCORES * NS, D), np.float32)
    x_pad[pos] = x
    shared = dict(
        fc_in_w=np.ascontiguousarray(fc_in_w, dtype=np.float32),
        fc_in_bT=np.asarray(fc_in_b, np.float32).reshape(D, 1),
        bn_g=np.asarray(bn_gamma, np.float32).reshape(1, D),
        bn_b=np.asarray(bn_beta, np.float32).reshape(1, D),
        gc_w=np.ascontiguousarray(
            np.asarray(gc_w, np.float32).reshape(L * D, D)
        ).astype(ml_dtypes.bfloat16),
        fc_out_w=np.ascontiguousarray(
            np.asarray(fc_out_w, np.float32)).astype(ml_dtypes.bfloat16),
        fc_out_bT=np.asarray(fc_out_b, np.float32).reshape(C, 1),
    )
    in_maps = []
    for c in range(NCORES):
        m = dict(shared)
        m["xt_own"] = np.ascontiguousarray(
            x_pad[c * NS:(c + 1) * NS].T)
        m.update(per_core[c])
        in_maps.append(m)

    res = run_bass_kernel_spmd(nc, in_maps, list(range(NCORES)),
                               trace=TRACE)
    LAST_EXEC_NS = res.exec_time_ns
    LAST_TRACE = res.instructions_and_trace

    out_full = np.zeros((N, C), np.float32)
    for c in range(NCORES):
        rows = res.results[c]["out"]
        nodes = pos2node[c * NS:(c + 1) * NS]
        v = nodes >= 0
        out_full[nodes[v]] = rows[v]
    return out_full


# revision 4
# speedup vs baseline: 1.0157x; 1.0157x over previous
"""DeepGCN (4-layer GCN, N=50000, E=800000, D=128) on 8 Trainium2 cores.

v3 = v2 (feature-major, bf16 spmm, DRAM-streamed one-hot M, Shared
AllGather) plus:
 - LPT edge balancing: max edges per (core, slot) <= 2048 -> K=16 chunks.
 - Pair indexing: gather 2-node pairs (idx = pos//2 < 25088 fits int16,
   no lo/hi split); each chunk does 2 matmuls (even/odd source parity).
 - prepare_only gathers + per-2-slot trigger_dma: descriptor generation
   (the serial GpSimd bottleneck) runs ahead across the inter-layer
   AllReduce/AllGather stalls.  The Gt pool WAR dependency (bufs=6)
   bounds prep-ahead to 12 x 1024 descriptors < the 16K SWDGE ring.
"""

import heapq
import sys

sys.path.insert(0, "/opt/trn_rl_repo")

import ml_dtypes
import numpy as np

import concourse.bacc as bacc
import concourse.mybir as mybir
import concourse.tile as tile
from concourse.bass_utils import run_bass_kernel_spmd
from concourse.library_config import mlp
from concourse.masks import make_identity

P = 128
NCORES = 8
N = 50000
D = 128
C = 40
L = 4
SLOTS = 49
NS = SLOTS * P
NTOT = NCORES * NS
NPAIR = NTOT // 2
MAXCH = 8
GAHEAD = 5              # Gt pool bufs = gather prep-ahead bound (slots)
EPS_BN = 1e-5
EPS_PN = 1e-6
GCH = 512
NGC = NS // GCH
GREM = NS - NGC * GCH

F32 = mybir.dt.float32
BF16 = mybir.dt.bfloat16
I16 = mybir.dt.int16

TRACE = False
LAST_EXEC_NS = None
LAST_TRACE = None
DEBUG_DUMP = False
PREP = False

_nc_cache = {}


# ------------------------------------------------------------------ host prep

def _positions(edge_row):
    """LPT-balance edge count over the 392 (core, slot) groups, 128 nodes
    per group cap."""
    deg = np.bincount(edge_row, minlength=N).astype(np.int64)
    order = np.argsort(-deg, kind="stable")
    NG = NCORES * SLOTS
    gsum = np.zeros(NG, np.int64)
    gcnt = np.zeros(NG, np.int64)
    heap = [(0, g) for g in range(NG)]
    heapq.heapify(heap)
    assign = np.empty(N, np.int64)
    for nd in order:
        while True:
            _, g = heapq.heappop(heap)
            if gcnt[g] < P:
                break
        assign[nd] = g
        gcnt[g] += 1
        gsum[g] += deg[nd]
        if gcnt[g] < P:
            heapq.heappush(heap, (gsum[g], g))

    pos = np.empty(N, np.int64)
    off = np.zeros(NG, np.int64)
    for nd in range(N):
        g = assign[nd]
        core, slot = g // SLOTS, g % SLOTS
        pos[nd] = core * NS + slot * P + off[g]
        off[g] += 1

    pos2node = np.full(NTOT, -1, np.int64)
    pos2node[pos] = np.arange(N)
    return pos, pos2node


def _preprocess(edge_row, edge_col, edge_val):
    pos, pos2node = _positions(edge_row)
    pd = pos[edge_row]
    ps = pos[edge_col]
    core = pd // NS
    slotg = (pd % NS) // P
    doff = pd % P
    pair = ps // 2
    par = ps % 2

    key = core * SLOTS + slotg
    cnt = np.bincount(key, minlength=NCORES * SLOTS).reshape(NCORES, SLOTS)
    K = np.ceil(cnt.max(axis=0) / P).astype(int)
    base = np.concatenate([[0], np.cumsum(K)[:-1]])
    TOT = int(K.sum())

    per_core = []
    for c in range(NCORES):
        sel = np.flatnonzero(core == c)
        k = slotg[sel]
        si = np.argsort(k, kind="stable")
        es = sel[si]
        ks = k[si]
        m = len(es)
        change = np.r_[True, np.diff(ks) != 0]
        segstart = np.maximum.accumulate(np.where(change, np.arange(m), 0))
        rank = np.arange(m) - segstart

        m_full = np.zeros((P, TOT * 2 * P), np.float32)
        idx_flat = np.zeros(TOT * P, np.int16)

        sl = ks
        gch = base[sl] + rank // P
        col = (gch * 2 + par[es]) * P + doff[es]
        m_full[rank % P, col] = edge_val[es]
        idx_flat[gch * P + rank % P] = pair[es]

        a = idx_flat.reshape(TOT * 8, 16).T
        idx_w = np.ascontiguousarray(np.tile(a, (8, 1)))

        per_core.append(dict(
            m_full=m_full.astype(ml_dtypes.bfloat16),
            idx=idx_w,
        ))

    sched = tuple(int(x) for x in K)
    meta = dict(K=K, base=base, TOT=TOT)
    return pos, pos2node, per_core, sched, meta


# ------------------------------------------------------------------ bass build

def _build(meta):
    K, base = meta["K"], meta["base"]
    TOT = meta["TOT"]
    OP = mybir.AluOpType
    AF = mybir.ActivationFunctionType
    AX = mybir.AxisListType
    RG = [list(range(NCORES))]

    nc = bacc.Bacc("TRN2", target_bir_lowering=False, debug=False,
                   num_devices=NCORES)

    xt_own = nc.dram_tensor("xt_own", [P, NS], F32, kind="ExternalInput")
    idx = nc.dram_tensor("idx", [P, TOT * 8], I16, kind="ExternalInput")
    m_full = nc.dram_tensor("m_full", [P, TOT * 2 * P], BF16,
                            kind="ExternalInput")
    fc_in_w = nc.dram_tensor("fc_in_w", [D, D], F32, kind="ExternalInput")
    fc_in_bT = nc.dram_tensor("fc_in_bT", [D, 1], F32, kind="ExternalInput")
    bn_g = nc.dram_tensor("bn_g", [1, D], F32, kind="ExternalInput")
    bn_b = nc.dram_tensor("bn_b", [1, D], F32, kind="ExternalInput")
    gc_w = nc.dram_tensor("gc_w", [L * D, D], BF16, kind="ExternalInput")
    fc_out_w = nc.dram_tensor("fc_out_w", [D, C], BF16, kind="ExternalInput")
    fc_out_bT = nc.dram_tensor("fc_out_bT", [C, 1], F32,
                               kind="ExternalInput")
    out = nc.dram_tensor("out", [NS, C], F32, kind="ExternalOutput")
    if DEBUG_DUMP:
        dbg_xn = nc.dram_tensor("dbg_xn", [P, NS], F32, kind="ExternalOutput")
        dbg_h = nc.dram_tensor("dbg_h", [P, NS], F32, kind="ExternalOutput")
        dbg_z = nc.dram_tensor("dbg_z", [P, NS], BF16, kind="ExternalOutput")

    gsem = nc.alloc_semaphore("gsem")

    with tile.TileContext(nc) as tc:
        nc.gpsimd.load_library(mlp)
        if PREP:
            nc.gpsimd.sem_clear(gsem)
        with (
            tc.tile_pool(name="const", bufs=1) as cp,
            tc.tile_pool(name="meta", bufs=1) as mp_,
            tc.tile_pool(name="big", bufs=1) as bp,
            tc.tile_pool(name="gpool", bufs=GAHEAD) as gp,
            tc.tile_pool(name="mpool", bufs=3) as mpl,
            tc.tile_pool(name="small", bufs=1) as sp,
            tc.tile_pool(name="dram", bufs=1, space="DRAM") as dp,
        ):
            # ---------------- constants
            ident = cp.tile([P, P], F32)
            make_identity(nc, ident[:])
            ones_col = cp.tile([P, 1], F32)
            nc.vector.memset(ones_col[:], 1.0)
            ones_row = cp.tile([1, P], F32)
            nc.vector.memset(ones_row[:], 1.0)
            eps_bn_t = cp.tile([P, 1], F32)
            nc.vector.memset(eps_bn_t[:], EPS_BN)
            eps_pn_t = cp.tile([1, 1], F32)
            nc.vector.memset(eps_pn_t[:], EPS_PN)

            w1_raw = cp.tile([D, D], F32)
            nc.sync.dma_start(w1_raw[:], fc_in_w[:])
            fcbT_s = cp.tile([D, 1], F32)
            nc.sync.dma_start(fcbT_s[:], fc_in_bT[:])
            bn_s = cp.tile([2, D], F32)
            nc.sync.dma_start(bn_s[0:1, :], bn_g[:])
            nc.sync.dma_start(bn_s[1:2, :], bn_b[:])
            gw_s = [cp.tile([D, D], BF16, tag=f"gw{i}", name=f"gw{i}")
                    for i in range(L)]
            for i in range(L):
                nc.sync.dma_start(gw_s[i][:], gc_w[i * D:(i + 1) * D, :])
            wo_s = cp.tile([D, C], BF16)
            nc.sync.dma_start(wo_s[:], fc_out_w[:])
            boT_s = cp.tile([C, 1], F32)
            nc.sync.dma_start(boT_s[:], fc_out_bT[:])

            idx_s = mp_.tile([P, TOT * 8], I16)
            nc.sync.dma_start(idx_s[:], idx[:])

            # ---------------- big SBUF tiles (feature-major)
            hsb = bp.tile([P, NS], F32)
            xn = bp.tile([P, NS], F32)
            rbuf = bp.tile([P, NS], F32)
            zb = bp.tile([P, NS], BF16)
            stage = bp.tile([P, SLOTS, P], BF16)

            # DRAM internals (Shared: one writer each, read by all cores)
            X_t = [dp.tile([NPAIR, 2 * P], BF16, addr_space="Shared",
                           tag=f"X{i}", name=f"X{i}")
                   for i in range(L)]
            ag_in = dp.tile([NS, P], BF16)
            st_in = dp.tile([P, 2], F32)
            st_out = dp.tile([P, 2], F32)

            def row_major_writeback(src_big, psp, tag):
                for s in range(SLOTS):
                    tp = psp.tile([P, P], F32, space="PSUM", tag=tag)
                    nc.tensor.transpose(
                        tp[:], src_big[:, s * P:(s + 1) * P], ident[:])
                    nc.scalar.copy(stage[:, s, :], tp[:])
                nc.sync.dma_start(
                    ag_in[:].rearrange("(s d) f -> d s f", s=SLOTS),
                    stage[:])

            # ---------------- phase 0: BN stats + folded fc_in
            with (
                tc.tile_pool(name="p0psum", bufs=2, space="PSUM") as pp0,
                tc.tile_pool(name="p0sb", bufs=1) as sp0,
            ):
                xt_s = sp0.tile([P, NS], F32)
                nc.sync.dma_start(xt_s[:], xt_own[:])

                colsum_o = sp0.tile([P, 1], F32)
                sumsq_o = sp0.tile([P, 1], F32)
                nc.vector.tensor_reduce(colsum_o[:], xt_s[:], axis=AX.X,
                                        op=OP.add)
                nc.scalar.activation(hsb[:], xt_s[:], AF.Square,
                                     accum_out=sumsq_o[:])
                st2 = sp0.tile([P, 2], F32)
                nc.vector.tensor_copy(st2[:, 0:1], colsum_o[:])
                nc.vector.tensor_copy(st2[:, 1:2], sumsq_o[:])
                nc.sync.dma_start(st_in[:], st2[:])
                nc.gpsimd.collective_compute(
                    "AllReduce", OP.add, replica_groups=RG,
                    ins=[st_in[:]], outs=[st_out[:]])
                stg = sp0.tile([P, 2], F32)
                nc.sync.dma_start(stg[:], st_out[:])

                mu = sp0.tile([P, 1], F32)
                nc.vector.tensor_scalar_mul(mu[:], stg[:, 0:1], 1.0 / N)
                msq = sp0.tile([P, 1], F32)
                nc.vector.tensor_scalar_mul(msq[:], stg[:, 1:2], 1.0 / N)
                mu2 = sp0.tile([P, 1], F32)
                nc.vector.tensor_tensor(mu2[:], mu[:], mu[:], op=OP.mult)
                var = sp0.tile([P, 1], F32)
                nc.vector.tensor_tensor(var[:], msq[:], mu2[:],
                                        op=OP.subtract)
                sd = sp0.tile([P, 1], F32)
                nc.scalar.activation(sd[:], var[:], AF.Sqrt,
                                     bias=eps_bn_t[:])
                rs = sp0.tile([P, 1], F32)
                nc.vector.reciprocal(rs[:], sd[:])

                bnT_ps = pp0.tile([P, 2], F32, space="PSUM", tag="pp0a")
                nc.tensor.transpose(bnT_ps[:], bn_s[:], ident[:2, :2])
                bnT = sp0.tile([P, 2], F32)
                nc.scalar.copy(bnT[:], bnT_ps[:])
                a_t = sp0.tile([P, 1], F32)
                nc.vector.tensor_tensor(a_t[:], bnT[:, 0:1], rs[:],
                                        op=OP.mult)
                t2 = sp0.tile([P, 1], F32)
                nc.vector.tensor_tensor(t2[:], mu[:], a_t[:], op=OP.mult)
                csh = sp0.tile([P, 1], F32)
                nc.vector.tensor_tensor(csh[:], bnT[:, 1:2], t2[:],
                                        op=OP.subtract)
                W1f = sp0.tile([D, D], F32)
                nc.vector.tensor_scalar_mul(W1f[:], w1_raw[:], a_t[:])
                b1_ps = pp0.tile([P, 1], F32, space="PSUM", tag="pp0a")
                nc.tensor.matmul(b1_ps[:], lhsT=w1_raw[:], rhs=csh[:],
                                 start=True, stop=True)
                b1T = sp0.tile([P, 1], F32)
                nc.scalar.copy(b1T[:], b1_ps[:])
                nc.vector.tensor_tensor(b1T[:], b1T[:], fcbT_s[:], op=OP.add)

                for j in range(NGC + 1):
                    c0 = j * GCH
                    w = GCH if j < NGC else GREM
                    if w == 0:
                        break
                    g_ps = pp0.tile([P, GCH], F32, space="PSUM", tag="g0")
                    nc.tensor.matmul(g_ps[:, :w], lhsT=W1f[:],
                                     rhs=xt_s[:, c0:c0 + w],
                                     start=True, stop=True)
                    nc.vector.tensor_scalar(
                        out=xn[:, c0:c0 + w], in0=g_ps[:, :w],
                        scalar1=b1T[:], scalar2=None, op0=OP.add)
                row_major_writeback(xn, pp0, "g0")
                nc.gpsimd.collective_compute(
                    "AllGather", OP.bypass, replica_groups=RG,
                    ins=[ag_in[:]], outs=[X_t[0][:]])

            # ---------------- layers
            prep_count = 0
            for li in range(L):
                XIN = X_t[li]
                with (
                    tc.tile_pool(name=f"l{li}ps", bufs=2, space="PSUM") as lp,
                    tc.tile_pool(name=f"l{li}hp", bufs=2, space="PSUM") as hp,
                ):
                    npend = 0
                    for s in range(SLOTS):
                        ks = int(K[s])
                        Gt = gp.tile([P, ks, 2 * P], BF16, tag="G")
                        for b0 in range(0, ks, MAXCH):
                            kk = min(MAXCH, ks - b0)
                            c0 = int(base[s]) + b0
                            if PREP:
                                nc.gpsimd.dma_gather(
                                    Gt[:, b0:b0 + kk, :], XIN[:],
                                    idx_s[:, c0 * 8:(c0 + kk) * 8],
                                    kk * P, kk * P, 2 * P,
                                    prepare_only=True, sem=gsem)
                                prep_count += 1
                            else:
                                nc.gpsimd.dma_gather(
                                    Gt[:, b0:b0 + kk, :], XIN[:],
                                    idx_s[:, c0 * 8:(c0 + kk) * 8],
                                    kk * P, kk * P, 2 * P)
                        if PREP:
                            nc.gpsimd.trigger_dma(count=None)
                            nc.tensor.wait_ge(gsem, 16 * prep_count)

                        Mt = mpl.tile([P, 2 * ks, P], BF16, tag="M")
                        mc0 = int(base[s])
                        nc.sync.dma_start(
                            Mt[:], m_full[:, mc0 * 2 * P:(mc0 + ks) * 2 * P])

                        zT = lp.tile([P, P], F32, space="PSUM", tag="zT")
                        for k in range(ks):
                            for par in (0, 1):
                                nc.tensor.matmul(
                                    zT[:],
                                    lhsT=Gt[:, k, par * P:(par + 1) * P],
                                    rhs=Mt[:, k * 2 + par, :],
                                    start=(k == 0 and par == 0),
                                    stop=(k == ks - 1 and par == 1))
                        if s % 2 == 0:
                            nc.scalar.copy(zb[:, s * P:(s + 1) * P], zT[:])
                        else:
                            nc.vector.tensor_copy(
                                zb[:, s * P:(s + 1) * P], zT[:])

                    # hT = W_l^T @ zb
                    for j in range(NGC + 1):
                        c0 = j * GCH
                        w = GCH if j < NGC else GREM
                        if w == 0:
                            break
                        h_ps = hp.tile([P, GCH], F32, space="PSUM", tag="h")
                        nc.tensor.matmul(h_ps[:, :w], lhsT=gw_s[li][:],
                                         rhs=zb[:, c0:c0 + w],
                                         start=True, stop=True)
                        nc.scalar.copy(hsb[:, c0:c0 + w], h_ps[:, :w])

                    # PairNorm stats
                    colsum = sp.tile([P, 1], F32, tag="colsum")
                    nc.vector.tensor_reduce(colsum[:], hsb[:], axis=AX.X,
                                            op=OP.add)
                    sumsq = sp.tile([P, 1], F32, tag="sumsq")
                    nc.scalar.activation(rbuf[:], hsb[:], AF.Square,
                                         accum_out=sumsq[:])
                    st2l = sp.tile([P, 2], F32, tag="st2l")
                    nc.vector.tensor_copy(st2l[:, 0:1], colsum[:])
                    nc.vector.tensor_copy(st2l[:, 1:2], sumsq[:])
                    nc.sync.dma_start(st_in[:], st2l[:])
                    nc.gpsimd.collective_compute(
                        "AllReduce", OP.add, replica_groups=RG,
                        ins=[st_in[:]], outs=[st_out[:]])
                    stgl = sp.tile([P, 2], F32, tag="stgl")
                    nc.sync.dma_start(stgl[:], st_out[:])

                    cmean = sp.tile([P, 1], F32, tag="cmean")
                    nc.vector.tensor_scalar_mul(cmean[:], stgl[:, 0:1],
                                                1.0 / N)
                    csq = sp.tile([P, 1], F32, tag="csq")
                    nc.vector.tensor_tensor(csq[:], stgl[:, 0:1],
                                            stgl[:, 0:1], op=OP.mult)
                    nc.vector.tensor_scalar_mul(csq[:], csq[:], 1.0 / N)
                    q = sp.tile([P, 1], F32, tag="q")
                    nc.vector.tensor_tensor(q[:], stgl[:, 1:2], csq[:],
                                            op=OP.subtract)
                    tot_ps = lp.tile([1, 1], F32, space="PSUM", tag="tot")
                    nc.tensor.matmul(tot_ps[:], lhsT=q[:], rhs=ones_col[:],
                                     start=True, stop=True)
                    tot_s = sp.tile([1, 1], F32, tag="tot")
                    nc.scalar.copy(tot_s[:], tot_ps[:])
                    rn = sp.tile([1, 1], F32, tag="rn")
                    nc.scalar.activation(rn[:], tot_s[:], AF.Sqrt,
                                         bias=eps_pn_t[:], scale=1.0 / N)
                    sres = sp.tile([1, 1], F32, tag="sres")
                    nc.vector.reciprocal(sres[:], rn[:])
                    sbc_ps = lp.tile([P, 1], F32, space="PSUM", tag="sbc")
                    nc.tensor.matmul(sbc_ps[:], lhsT=ones_row[:],
                                     rhs=sres[:], start=True, stop=True)
                    sbc = sp.tile([P, 1], F32, tag="sbc")
                    nc.scalar.copy(sbc[:], sbc_ps[:])
                    nsm = sp.tile([P, 1], F32, tag="nsm")
                    nc.vector.tensor_tensor(nsm[:], cmean[:], sbc[:],
                                            op=OP.mult)
                    nc.vector.tensor_scalar_mul(nsm[:], nsm[:], -1.0)

                    # pass 2 (x_old starts at zero: no residual at layer 0)
                    # chunked by slot-quarters so the row-major writeback
                    # (transposes + copies + DMA) pipelines under it
                    bounds = [0, 13, 25, 37, 49]
                    agv = ag_in[:].rearrange("(s d) f -> d s f", s=SLOTS)
                    for qi in range(4):
                        s0, s1 = bounds[qi], bounds[qi + 1]
                        c0, c1 = s0 * P, s1 * P
                        if li == 0:
                            nc.scalar.activation(
                                xn[:, c0:c1], hsb[:, c0:c1], AF.Relu,
                                scale=sbc[:], bias=nsm[:])
                        else:
                            nc.scalar.activation(
                                rbuf[:, c0:c1], hsb[:, c0:c1], AF.Relu,
                                scale=sbc[:], bias=nsm[:])
                            nc.vector.tensor_tensor(
                                xn[:, c0:c1], rbuf[:, c0:c1], xn[:, c0:c1],
                                op=OP.add)
                        if li < L - 1:
                            for s in range(s0, s1):
                                tp = lp.tile([P, P], F32, space="PSUM",
                                             tag="zT")
                                nc.tensor.transpose(
                                    tp[:], xn[:, s * P:(s + 1) * P],
                                    ident[:])
                                if s % 2 == 0:
                                    nc.scalar.copy(stage[:, s, :], tp[:])
                                else:
                                    nc.vector.tensor_copy(
                                        stage[:, s, :], tp[:])
                            nc.sync.dma_start(
                                agv[:, s0:s1, :], stage[:, s0:s1, :])

                    if li < L - 1:
                        nc.gpsimd.collective_compute(
                            "AllGather", OP.bypass, replica_groups=RG,
                            ins=[ag_in[:]], outs=[X_t[li + 1][:]])

            if DEBUG_DUMP:
                nc.sync.dma_start(dbg_xn[:], xn[:])
                nc.sync.dma_start(dbg_h[:], hsb[:])
                nc.sync.dma_start(dbg_z[:], zb[:])

            # ---------------- fc_out (reuse zb as the bf16 activations)
            with (
                tc.tile_pool(name="fo", bufs=3, space="PSUM") as fp,
                tc.tile_pool(name="fos", bufs=2) as fs,
            ):
                nc.vector.tensor_copy(zb[:], xn[:])
                for j in range(NGC + 1):
                    c0 = j * GCH
                    w = GCH if j < NGC else GREM
                    if w == 0:
                        break
                    o_ps = fp.tile([C, GCH], F32, space="PSUM", tag="o")
                    nc.tensor.matmul(o_ps[:, :w], lhsT=wo_s[:],
                                     rhs=zb[:, c0:c0 + w],
                                     start=True, stop=True)
                    oT = fs.tile([C, GCH], F32, tag="oT")
                    nc.vector.tensor_scalar(
                        out=oT[:, :w], in0=o_ps[:, :w],
                        scalar1=boT_s[:], scalar2=None, op0=OP.add)
                    for dd in range(w // P):
                        s = (c0 + dd * P) // P
                        tp_ps = fp.tile([P, C], F32, space="PSUM", tag="tp")
                        nc.tensor.transpose(
                            tp_ps[:], oT[:, dd * P:(dd + 1) * P],
                            ident[:C, :C])
                        o_s = fs.tile([P, C], F32, tag="os")
                        nc.scalar.copy(o_s[:], tp_ps[:])
                        nc.sync.dma_start(out[s * P:(s + 1) * P, :], o_s[:])

    nc.compile()
    return nc


# ------------------------------------------------------------------ kernel

def kernel(x, edge_row, edge_col, edge_val, bn_gamma, bn_beta,
           fc_in_w, fc_in_b, gc_w, gc_b, fc_out_w, fc_out_b):
    global LAST_EXEC_NS, LAST_TRACE
    x = np.asarray(x, np.float32)
    edge_row = np.asarray(edge_row).astype(np.int64)
    edge_col = np.asarray(edge_col).astype(np.int64)
    edge_val = np.asarray(edge_val, np.float32)

    pos, pos2node, per_core, sched, meta = _preprocess(
        edge_row, edge_col, edge_val)

    if sched not in _nc_cache:
        _nc_cache[sched] = _build(meta)
    nc = _nc_cache[sched]

    x_pad = np.zeros((NTOT, D), np.float32)
    x_pad[pos] = x
    shared = dict(
        fc_in_w=np.ascontiguousarray(fc_in_w, dtype=np.float32),
        fc_in_bT=np.asarray(fc_in_b, np.float32).reshape(D, 1),
        bn_g=np.asarray(bn_gamma, np.float32).reshape(1, D),
        bn_b=np.asarray(bn_beta, np.float32).reshape(1, D),
        gc_w=np.ascontiguousarray(
            np.asarray(gc_w, np.float32).reshape(L * D, D)
        ).astype(ml_dtypes.bfloat16),
        fc_out_w=np.ascontiguousarray(
            np.asarray(fc_out_w, np.float32)).astype(ml_dtypes.bfloat16),
        fc_out_bT=np.asarray(fc_out_b, np.float32).reshape(C, 1),
    )
    in_maps = []
    for c in range(NCORES):
        m = dict(shared)
        m["xt_own"] = np.ascontiguousarray(
            x_pad[c * NS:(c + 1) * NS].T)
        m.update(per_core[c])
        in_maps.append(m)

    res = run_bass_kernel_spmd(nc, in_maps, list(range(NCORES)),
                               trace=TRACE)
    LAST_EXEC_NS = res.exec_time_ns
    LAST_TRACE = res.instructions_and_trace

    out_full = np.zeros((N, C), np.float32)
    for c in range(NCORES):
        rows = res.results[c]["out"]
        nodes = pos2node[c * NS:(c + 1) * NS]
        v = nodes >= 0
        out_full[nodes[v]] = rows[v]
    return out_full
